# revision 20
# baseline (speedup 1.0000x reference)
"""GQA attention (16 q heads / 4 kv heads, HD=128, S=4096, D=2048) with RoPE,
causal mask, log-gate on kv positions, softmax, and output projection —
distributed over 8 NeuronCores.

Sharding: head-parallel. Core c computes q heads {2c, 2c+1} and kv head c//2.
Each core produces a partial [S, D] output (its 2 heads' contribution through
Wo, stored bf16) and the host sums the 8 partials.

On-device strategy (fp8 + exponent-bit-trick softmax):
 - Bulk matmuls run fp8 e4m3 with MatmulPerfMode.DoubleRow (2 contraction rows
   per partition -> 0.5 cycles/row): Q/K/V projections, Q.K^T scores, attn@V,
   softmax denominator, and the output projection.
 - Scores are computed pre-scaled by A = 4/ln2 so the PSUM value is directly
   the e5m2 EXPONENT-BIT code of exp(s)*gate: softmax exp() becomes a single
   clamp op (min 123, max 0) writing int8, bitcast to fp8 e5m2
   (value = 2^((bits-60)/4)). The per-key bias 4*log2(gate) + B0 rides in the
   scores matmul itself as a 65th contraction partition (hi+lo e4m3 split).
   The Act engine instead computes true exp(psum*ln2/4 - 60*ln2/4) -> e5m2,
   numerically consistent with the bit-trick; exp work is split across
   Act/DVE(/Pool) engines. Diagonal blocks fuse causal masking into the clamp:
   (psum max 0) * mask01 -> int8.
 - Rows 0..511 (the large-magnitude early rows) use a bf16 path end-to-end:
   bf16 projections, bf16 scores + additive -1e30 mask, Act true exp -> bf16,
   bf16 attn@V and output projection.
 - V is scaled by 16 and Wo by 32 to dodge e4m3 subnormals; undone in the
   final PSUM->SBUF copies.
"""

import math
from contextlib import ExitStack

import numpy as np

import concourse.bass as bass
import concourse.mybir as mybir
import concourse.tile as tile
from concourse import bacc
from concourse._compat import with_exitstack
from concourse.bass import ds
from concourse.bass_utils import run_bass_kernel_spmd
from concourse.masks import make_identity

P = 128
F = 512            # q-chunk width
S = 4096
D = 2048
HD = 128
NB = S // F        # 8 chunks
NJB = S // P       # 32 key blocks
KOP = D // 256     # 8 DoubleRow contraction pairs for projections
F32 = mybir.dt.float32
BF16 = mybir.dt.bfloat16
FP8E4 = mybir.dt.float8e4
FP8E5 = mybir.dt.float8e5
I8 = mybir.dt.int8
DRM = mybir.MatmulPerfMode.DoubleRow
ExpF = mybir.ActivationFunctionType.Exp
CopyF = mybir.ActivationFunctionType.Copy
AL = mybir.AluOpType

GR = 4.0                                # e5m2 bits per octave
OFF = 60.0                              # e5m2 bits of 1.0
B0 = 66.0                               # recentering constant
CLIP = 123.0                            # max finite e5m2 bits
A_SCALE = GR / math.log(2.0)            # score prescale (in q tables)
ACT_SCALE = math.log(2.0) / GR
ACT_BIAS = -OFF * math.log(2.0) / GR
VSCALE = 16.0
WOSCALE = 32.0

# engine rotation patterns (A=Act, D=DVE, P=Pool/gpsimd)
EXP_PATTERN = "AD"         # non-diagonal fp8 exp ops (PSUM -> Act/DVE only)
DIAG_PATTERN = "DD"        # diagonal fp8 exp ops (tensor mask -> DVE)
OUT_PATTERN = "AAD"        # outproj psum->sbuf copies (PSUM -> Act/DVE only)


@with_exitstack
def _body(ctx: ExitStack, tc: tile.TileContext, io: dict):
    nc = tc.nc

    persist = ctx.enter_context(tc.tile_pool(name="persist", bufs=1))
    qT8 = persist.tile([65, 2, 2, S], FP8E4, tag="qT8")      # [p, t, h, i]
    kT8 = persist.tile([65, 2, S], FP8E4, tag="kT8")         # [p, t, j]
    vv8 = persist.tile([P, NJB // 2, 2, HD], FP8E4, tag="vv8")   # [j, jp, t, d]
    attnT8 = persist.tile([P, 2, S], FP8E4, tag="attnT8")    # [d, h, i]
    q16t = persist.tile([P, 2, F], BF16, tag="q16t")         # chunk-0 q
    kT16 = persist.tile([P, F], BF16, tag="kT16")            # chunk-0 k
    vv16 = persist.tile([P, 4, HD], BF16, tag="vv16")        # chunk-0 v
    attnT16 = persist.tile([P, 2, F], BF16, tag="attnT16")
    dmask = persist.tile([P, 4, F], F32, tag="dmask")
    m01 = persist.tile([P, 4, F], F32, tag="m01")
    bias0 = persist.tile([P, 4], F32, tag="bias0")
    ident32 = persist.tile([P, P], F32, tag="ident32")
    ones8 = persist.tile([P, 2, P], FP8E4, tag="ones8")
    ones16 = persist.tile([P, P], BF16, tag="ones16")
    abias = persist.tile([P, 1], F32, tag="abias")

    nc.sync.dma_start(dmask[:], io["dmask"])
    nc.sync.dma_start(m01[:], io["m01"])
    nc.sync.dma_start(bias0[:], io["bias0"])
    nc.sync.dma_start(kT8[64:65, :, :], io["brow"])
    nc.sync.dma_start(qT8[64:65, :, :, :], io["qones"])
    make_identity(nc, ident32[:])
    nc.vector.memset(ones8[:], 1.0)
    nc.vector.memset(ones16[:], 1.0)
    nc.vector.memset(abias[:], ACT_BIAS)

    wpool = ctx.enter_context(tc.tile_pool(name="wpool", bufs=1))
    wq8 = wpool.tile([P, KOP, 2, 256], FP8E4, tag="wq8")
    wk8 = wpool.tile([P, KOP, 2, HD], FP8E4, tag="wk8")
    wv8 = wpool.tile([P, KOP, 2, HD], FP8E4, tag="wv8")
    wq16 = wpool.tile([P, 16, 256], BF16, tag="wq16")
    wk16 = wpool.tile([P, 16, HD], BF16, tag="wk16")
    wv16 = wpool.tile([P, 16, HD], BF16, tag="wv16")
    wo8 = wpool.tile([P, 2, D], FP8E4, tag="wo8")
    wo16 = wpool.tile([P, 2, D], BF16, tag="wo16")
    xt16 = wpool.tile([P, 16, F], BF16, tag="xt16")
    for nm, t in [("wq8", wq8), ("wk8", wk8), ("wv8", wv8), ("wq16", wq16),
                  ("wk16", wk16), ("wv16", wv16), ("wo8", wo8), ("wo16", wo16),
                  ("xt16", xt16)]:
        nc.sync.dma_start(t[:], io[nm])

    xt_pool = ctx.enter_context(tc.tile_pool(name="xt", bufs=2))
    tab_pool = ctx.enter_context(tc.tile_pool(name="tab", bufs=2))
    rope_pool = ctx.enter_context(tc.tile_pool(name="rope", bufs=3))
    exp_pool = ctx.enter_context(tc.tile_pool(name="exp", bufs=4))
    bc_pool = ctx.enter_context(tc.tile_pool(name="bc", bufs=2))
    ob_pool = ctx.enter_context(tc.tile_pool(name="ob", bufs=2))
    psProj = ctx.enter_context(tc.tile_pool(name="psProj", bufs=1, space="PSUM"))
    psSc = ctx.enter_context(tc.tile_pool(name="psSc", bufs=4, space="PSUM"))
    psAv = ctx.enter_context(tc.tile_pool(name="psAv", bufs=1, space="PSUM"))
    psSum = ctx.enter_context(tc.tile_pool(name="psSum", bufs=1, space="PSUM"))
    psO = ctx.enter_context(tc.tile_pool(name="psO", bufs=1, space="PSUM"))

    rr = {"exp": 0, "diag": 0, "out": 0}

    def pick(kind, pattern):
        c = pattern[rr[kind] % len(pattern)]
        rr[kind] += 1
        return c

    def emit_exp_fp8(sc_ps, ex_slot, ex_slot_i8, dp):
        if dp >= 0:
            c = pick("diag", DIAG_PATTERN)
            eng = nc.vector if c == "D" else nc.gpsimd
            eng.scalar_tensor_tensor(ex_slot_i8, sc_ps, 0.0, m01[:, dp, :],
                                     op0=AL.max, op1=AL.mult)
        else:
            c = pick("exp", EXP_PATTERN)
            if c == "A":
                nc.scalar.activation(ex_slot, sc_ps, ExpF, bias=abias[:],
                                     scale=ACT_SCALE)
            else:
                eng = nc.vector if c == "D" else nc.gpsimd
                eng.tensor_scalar(ex_slot_i8, sc_ps, CLIP, 0.0,
                                  op0=AL.min, op1=AL.max)

    def rope(ps, ct, st, dest_ap):
        tmp = rope_pool.tile([P, F], F32, tag="tmp")
        nc.scalar.copy(tmp[:], ps[:])
        rotd = rope_pool.tile([P, F], F32, tag="rot")
        nc.gpsimd.dma_start(rotd[0:64, :], tmp[64:128, :])
        nc.gpsimd.dma_start(rotd[64:128, :], tmp[0:64, :])
        r2 = rope_pool.tile([P, F], F32, tag="r2")
        nc.gpsimd.tensor_tensor(r2[:], rotd[:], st, op=AL.mult)
        t1 = rope_pool.tile([P, F], F32, tag="t1")
        nc.gpsimd.tensor_tensor(t1[:], tmp[:], ct, op=AL.mult)
        nc.gpsimd.tensor_tensor(dest_ap, t1[:], r2[:], op=AL.add)

    def vtrans(psv, vv8_dst, vv16_dst):
        """v psum [d, i] -> x16, transpose per 128-block -> vv8 (and vv16)."""
        vt32 = rope_pool.tile([P, F], F32, tag="vt32")
        nc.scalar.activation(vt32[:], psv[:], CopyF, scale=VSCALE)
        ptf = psSc.tile([P, F], F32, tag="sc")
        for dp in range(4):
            nc.tensor.transpose(ptf[:, ds(dp * P, P)], vt32[:, ds(dp * P, P)],
                                ident32[:])
        nc.scalar.copy(vv8_dst, ptf[:])
        if vv16_dst is not None:
            nc.scalar.copy(vv16_dst, ptf[:])

    pending = []

    def drain_one():
        while pending:
            try:
                next(pending[0])
                return True
            except StopIteration:
                pending.pop(0)
        return False

    def flush_pending():
        while drain_one():
            pass

    def proj8(xt8c, w_sb, m0, width):
        ps = psProj.tile([P, F], F32, tag="pp")
        for kop in range(KOP):
            nc.tensor.matmul(ps[:], lhsT=w_sb[:, kop, :, ds(m0, width)],
                             rhs=xt8c[:, kop, :, :],
                             start=(kop == 0), stop=(kop == KOP - 1),
                             perf_mode=DRM)
        return ps

    def proj16(w_sb, m0, width):
        ps = psProj.tile([P, F], F32, tag="pp")
        for ko in range(16):
            nc.tensor.matmul(ps[:], lhsT=w_sb[:, ko, ds(m0, width)],
                             rhs=xt16[:, ko, :],
                             start=(ko == 0), stop=(ko == 15))
        return ps

    def proj_items(nb):
        """Generator emitting q/k/v projection work for chunk nb."""
        def gen():
            sl = ds(nb * F, F)
            tabs = tab_pool.tile([P, 4, F], BF16, tag="tabs")
            nc.sync.dma_start(tabs[:], io["tabs"][:, :, sl])
            ctA, stA = tabs[:, 0, :], tabs[:, 1, :]
            ct, st = tabs[:, 2, :], tabs[:, 3, :]
            if nb == 0:
                # kv were done in the prologue; only bf16 q remains
                yield
                for h in range(2):
                    psq = proj16(wq16, h * P, P)
                    yield
                    rope(psq, ctA, stA, q16t[:, h, :])
                    yield
                return
            xt8c = xt_pool.tile([P, KOP, 2, F], FP8E4, tag="xt8c")
            nc.sync.dma_start(xt8c[:], io["xt8"][:, :, :, sl])
            yield
            for h in range(2):
                psq = proj8(xt8c, wq8, h * P, P)
                yield
                stq = rope_pool.tile([P, F], FP8E4, tag="stq8")
                rope(psq, ctA, stA, stq[:])
                nc.gpsimd.dma_start(qT8[0:64, 0, h, sl], stq[0:64, :])
                nc.gpsimd.dma_start(qT8[0:64, 1, h, sl], stq[64:128, :])
                yield
            psk = proj8(xt8c, wk8, 0, P)
            yield
            stk = rope_pool.tile([P, F], FP8E4, tag="stk8")
            rope(psk, ct, st, stk[:])
            nc.gpsimd.dma_start(kT8[0:64, 0, sl], stk[0:64, :])
            nc.gpsimd.dma_start(kT8[0:64, 1, sl], stk[64:128, :])
            yield
            psv = proj8(xt8c, wv8, 0, P)
            yield
            vtrans(psv, vv8[:, nb * 2: nb * 2 + 2, :, :], None)
            yield
        return gen()

    def outproj_items(nb):
        for ib in range(4):
            i2 = nb * 4 + ib
            ob = ob_pool.tile([P, D], BF16, tag="ob")

            def mk(i2=i2, ob=ob, first=(nb == 0)):
                def items():
                    for e in range(4):
                        po = psO.tile([P, F], F32, tag="po")
                        if first:
                            for hh in range(2):
                                nc.tensor.matmul(
                                    po[:], lhsT=attnT16[:, hh, ds((i2 % 4) * P, P)],
                                    rhs=wo16[:, hh, ds(e * F, F)],
                                    start=(hh == 0), stop=(hh == 1))
                            yield
                            nc.scalar.activation(ob[:, ds(e * F, F)], po[:],
                                                 CopyF, scale=1.0 / VSCALE)
                        else:
                            nc.tensor.matmul(
                                po[:], lhsT=attnT8[:, :, ds(i2 * P, P)],
                                rhs=wo8[:, :, ds(e * F, F)],
                                start=True, stop=True, perf_mode=DRM)
                            yield
                            c = pick("out", OUT_PATTERN)
                            sc = 1.0 / (VSCALE * WOSCALE)
                            if c == "A":
                                nc.scalar.activation(ob[:, ds(e * F, F)], po[:],
                                                     CopyF, scale=sc)
                            elif c == "D":
                                nc.vector.tensor_scalar_mul(
                                    ob[:, ds(e * F, F)], po[:], sc)
                            else:
                                nc.gpsimd.tensor_scalar_mul(
                                    ob[:, ds(e * F, F)], po[:], sc)
                        yield
                    nc.sync.dma_start(io["outp"][ds(i2 * P, P), :], ob[:])
                    yield
                return items()
            pending.append(mk())

    def attention_fp8(nb):
        sl = ds(nb * F, F)
        npair = (4 * nb + 4) // 2
        for h in range(2):
            av = psAv.tile([P, F], F32, tag="av")
            sm = psSum.tile([P, F], F32, tag="sm")

            def av_sum(ex_prev, jp_prev):
                nc.tensor.matmul(av[:], lhsT=vv8[:, jp_prev, :, :],
                                 rhs=ex_prev[:],
                                 start=(jp_prev == 0),
                                 stop=(jp_prev == npair - 1),
                                 perf_mode=DRM)
                nc.tensor.matmul(sm[:], lhsT=ones8[:], rhs=ex_prev[:],
                                 start=(jp_prev == 0),
                                 stop=(jp_prev == npair - 1),
                                 perf_mode=DRM)

            prev = None
            for jp in range(npair):
                ex = exp_pool.tile([P, 2, F], FP8E5, tag="ex")
                for t in range(2):
                    jb = 2 * jp + t
                    sc = psSc.tile([P, F], F32, tag="sc")
                    nc.tensor.matmul(sc[:], lhsT=kT8[:, :, ds(jb * P, P)],
                                     rhs=qT8[:, :, h, sl],
                                     start=True, stop=True, perf_mode=DRM)
                    emit_exp_fp8(sc[:], ex[:, t, :],
                                 ex[:, t, :].bitcast(I8), jb - 4 * nb)
                drain_one()
                drain_one()
                if prev is not None:
                    av_sum(*prev)
                prev = (ex, jp)
            av_sum(*prev)
            rrow = bc_pool.tile([1, F], F32, tag="rrow")
            nc.vector.reciprocal_approx_fast(rrow[:], sm[0:1, :])
            rbc = bc_pool.tile([P, F], F32, tag="rbc")
            nc.gpsimd.partition_broadcast(rbc[:], rrow[0:1, :])
            nc.vector.tensor_tensor(attnT8[:, h, sl], av[:], rbc[:],
                                    op=AL.mult)

    def attention_c0():
        for h in range(2):
            av = psAv.tile([P, F], F32, tag="av")
            sm = psSum.tile([P, F], F32, tag="sm")
            for jb in range(4):
                sc = psSc.tile([P, F], F32, tag="sc")
                nc.tensor.matmul(sc[:], lhsT=kT16[:, ds(jb * P, P)],
                                 rhs=q16t[:, h, :], start=True, stop=True)
                nc.vector.tensor_tensor(sc[:], sc[:], dmask[:, jb, :],
                                        op=AL.add)
                ex0 = exp_pool.tile([P, F], BF16, tag="ex0")
                nc.scalar.activation(ex0[:], sc[:], ExpF,
                                     bias=bias0[:, jb: jb + 1],
                                     scale=ACT_SCALE)
                nc.tensor.matmul(av[:], lhsT=vv16[:, jb, :], rhs=ex0[:],
                                 start=(jb == 0), stop=(jb == 3))
                nc.tensor.matmul(sm[:], lhsT=ones16[:], rhs=ex0[:],
                                 start=(jb == 0), stop=(jb == 3))
                drain_one()
            rrow = bc_pool.tile([1, F], F32, tag="rrow")
            nc.vector.reciprocal_approx_fast(rrow[:], sm[0:1, :])
            rbc = bc_pool.tile([P, F], F32, tag="rbc")
            nc.gpsimd.partition_broadcast(rbc[:], rrow[0:1, :])
            nc.vector.tensor_tensor(attnT16[:, h, :], av[:], rbc[:],
                                    op=AL.mult)

    # ── prologue: chunk-0 K/V (bf16 path; also feeds the fp8 tiles) ──
    tabs0 = tab_pool.tile([P, 4, F], BF16, tag="tabs")
    nc.sync.dma_start(tabs0[:], io["tabs"][:, :, ds(0, F)])
    psk = proj16(wk16, 0, P)
    rope(psk, tabs0[:, 2, :], tabs0[:, 3, :], kT16[:])
    stk8 = rope_pool.tile([P, F], FP8E4, tag="stk8")
    nc.vector.tensor_copy(stk8[:], kT16[:])
    nc.gpsimd.dma_start(kT8[0:64, 0, ds(0, F)], stk8[0:64, :])
    nc.gpsimd.dma_start(kT8[0:64, 1, ds(0, F)], stk8[64:128, :])
    psv = proj16(wv16, 0, P)
    vtrans(psv, vv8[:, 0:2, :, :], vv16[:])

    ORDER = [1, 2, 3, 4, 5, 6, 7, 0]
    pending.append(proj_items(ORDER[0]))
    for idx, nb in enumerate(ORDER):
        flush_pending()
        if idx + 1 < len(ORDER):
            pending.append(proj_items(ORDER[idx + 1]))
        if nb == 0:
            attention_c0()
        else:
            attention_fp8(nb)
        outproj_items(nb)
    flush_pending()


_NC_CACHE = None


def build_nc():
    global _NC_CACHE
    if _NC_CACHE is not None:
        return _NC_CACHE
    nc = bacc.Bacc("TRN2", target_bir_lowering=False, debug=False)
    io = {
        "xt8": nc.dram_tensor("xt8", [P, KOP, 2, S], FP8E4, kind="ExternalInput").ap(),
        "xt16": nc.dram_tensor("xt16", [P, 16, F], BF16, kind="ExternalInput").ap(),
        "tabs": nc.dram_tensor("tabs", [P, 4, S], BF16, kind="ExternalInput").ap(),
        "wq8": nc.dram_tensor("wq8", [P, KOP, 2, 256], FP8E4, kind="ExternalInput").ap(),
        "wk8": nc.dram_tensor("wk8", [P, KOP, 2, HD], FP8E4, kind="ExternalInput").ap(),
        "wv8": nc.dram_tensor("wv8", [P, KOP, 2, HD], FP8E4, kind="ExternalInput").ap(),
        "wq16": nc.dram_tensor("wq16", [P, 16, 256], BF16, kind="ExternalInput").ap(),
        "wk16": nc.dram_tensor("wk16", [P, 16, HD], BF16, kind="ExternalInput").ap(),
        "wv16": nc.dram_tensor("wv16", [P, 16, HD], BF16, kind="ExternalInput").ap(),
        "wo8": nc.dram_tensor("wo8", [P, 2, D], FP8E4, kind="ExternalInput").ap(),
        "wo16": nc.dram_tensor("wo16", [P, 2, D], BF16, kind="ExternalInput").ap(),
        "brow": nc.dram_tensor("brow", [1, 2, S], FP8E4, kind="ExternalInput").ap(),
        "qones": nc.dram_tensor("qones", [1, 2, 2, S], FP8E4, kind="ExternalInput").ap(),
        "bias0": nc.dram_tensor("bias0", [P, 4], F32, kind="ExternalInput").ap(),
        "dmask": nc.dram_tensor("dmask", [P, 4, F], F32, kind="ExternalInput").ap(),
        "m01": nc.dram_tensor("m01", [P, 4, F], F32, kind="ExternalInput").ap(),
        "outp": nc.dram_tensor("outp", [S, D], BF16, kind="ExternalOutput").ap(),
    }
    with tile.TileContext(nc) as tc:
        _body(tc, io)
    nc.compile()
    _NC_CACHE = nc
    return nc


def make_in_maps(hidden_states, attention_mask, cos, sin, gate, Wq, Wk, Wv, Wo):
    import ml_dtypes
    bf16 = ml_dtypes.bfloat16
    e4m3 = ml_dtypes.float8_e4m3
    f32 = np.float32

    X = np.asarray(hidden_states, f32).reshape(S, D)
    Xt = np.ascontiguousarray(X.T)                       # [D, S]
    xt8 = np.ascontiguousarray(
        Xt.reshape(KOP, 2, P, S).transpose(2, 0, 1, 3).astype(e4m3))
    xt16 = np.ascontiguousarray(
        X[:F].T.reshape(16, P, F).transpose(1, 0, 2).astype(bf16))

    cosT = np.asarray(cos, f32).reshape(S, HD).T         # [HD, S]
    sinT = np.asarray(sin, f32).reshape(S, HD).T
    sinTs = np.concatenate([-sinT[: HD // 2], sinT[HD // 2:]], axis=0)
    qs = f32(A_SCALE / math.sqrt(HD))
    tabs = np.ascontiguousarray(
        np.stack([cosT * qs, sinTs * qs, cosT, sinTs], axis=1).astype(bf16))

    g = np.asarray(gate, f32).reshape(S) + f32(1e-8)
    b = (B0 + GR * np.log2(g)).astype(f32)
    b_hi = b.astype(e4m3)
    b_lo = (b - b_hi.astype(f32)).astype(e4m3)
    brow = np.ascontiguousarray(np.stack([b_hi, b_lo], axis=0)[None])  # [1,2,S]
    qones = np.ones((1, 2, 2, S), dtype=e4m3)
    bias0 = np.ascontiguousarray(
        (np.log(g[:F]) + (B0 - OFF) * math.log(2.0) / GR)
        .reshape(4, P).T.astype(f32))                    # [128, 4]

    jj = np.arange(P)[:, None, None]
    pp = np.arange(4)[None, :, None] * P
    ii = np.arange(F)[None, None, :]
    valid = (jj + pp) <= ii
    dmask = np.ascontiguousarray(
        np.where(valid, f32(0), f32(-1e30)).astype(f32))
    m01 = np.ascontiguousarray(valid.astype(f32))

    Wq = np.asarray(Wq, f32); Wk = np.asarray(Wk, f32)
    Wv = np.asarray(Wv, f32); Wo = np.asarray(Wo, f32)

    def pack8(Wslice):               # [D, M] -> [128, KOP, 2, M]
        M = Wslice.shape[1]
        return np.ascontiguousarray(
            Wslice.reshape(KOP, 2, P, M).transpose(2, 0, 1, 3).astype(e4m3))

    def pack16(Wslice):              # [D, M] -> [128, 16, M]
        M = Wslice.shape[1]
        return np.ascontiguousarray(
            Wslice.reshape(16, P, M).transpose(1, 0, 2).astype(bf16))

    in_maps = []
    for c in range(8):
        gk = c // 2
        wq_sl = Wq[:, c * 256:(c + 1) * 256]
        wk_sl = Wk[:, gk * HD:(gk + 1) * HD]
        wv_sl = Wv[:, gk * HD:(gk + 1) * HD]
        wo_sl = Wo[c * 256:(c + 1) * 256, :]             # [256, D]
        wo8 = np.ascontiguousarray(
            (wo_sl * WOSCALE).reshape(2, P, D).transpose(1, 0, 2).astype(e4m3))
        wo16 = np.ascontiguousarray(
            wo_sl.reshape(2, P, D).transpose(1, 0, 2).astype(bf16))
        in_maps.append({
            "xt8": xt8, "xt16": xt16, "tabs": tabs,
            "wq8": pack8(wq_sl), "wk8": pack8(wk_sl), "wv8": pack8(wv_sl),
            "wq16": pack16(wq_sl), "wk16": pack16(wk_sl), "wv16": pack16(wv_sl),
            "wo8": wo8, "wo16": wo16,
            "brow": brow, "qones": qones, "bias0": bias0,
            "dmask": dmask, "m01": m01,
        })
    return in_maps


def kernel(hidden_states, attention_mask, cos, sin, gate, Wq, Wk, Wv, Wo,
           **kwargs):
    nc = build_nc()
    in_maps = make_in_maps(
        hidden_states, attention_mask, cos, sin, gate, Wq, Wk, Wv, Wo
    )
    res = run_bass_kernel_spmd(nc, in_maps, core_ids=list(range(8)), **kwargs)
    acc = res.results[0]["outp"].astype(np.float32)
    for c in range(1, 8):
        acc = acc + res.results[c]["outp"].astype(np.float32)
    out = acc.reshape(1, S, D)
    if kwargs:
        return out, res
    return out


# revision 21
# speedup vs baseline: 1.3895x; 1.3895x over previous
"""GQA attention (16 q heads / 4 kv heads, HD=128, S=4096, D=2048) with RoPE,
causal mask, log-gate on kv positions, softmax, and output projection —
distributed over 8 NeuronCores.

Sharding: head-parallel. Core c computes q heads {2c, 2c+1} and kv head c//2.
Each core produces a partial [S, D] output (its 2 heads' contribution through
Wo, stored bf16) and the host sums the 8 partials.

On-device strategy (fp8 + exponent-bit-trick softmax):
 - Bulk matmuls run fp8 e4m3 with MatmulPerfMode.DoubleRow (2 contraction rows
   per partition -> 0.5 cycles/row): Q/K/V projections, Q.K^T scores, attn@V,
   softmax denominator, and the output projection.
 - Scores are computed pre-scaled by A = 4/ln2 so the PSUM value is directly
   the e5m2 EXPONENT-BIT code of exp(s)*gate: softmax exp() becomes a single
   clamp op (min 123, max 0) writing int8, bitcast to fp8 e5m2
   (value = 2^((bits-60)/4)). The per-key bias 4*log2(gate) + B0 rides in the
   scores matmul itself as a 65th contraction partition (hi+lo e4m3 split).
   The Act engine instead computes true exp(psum*ln2/4 - 60*ln2/4) -> e5m2,
   numerically consistent with the bit-trick; exp work is split across
   Act/DVE(/Pool) engines. Diagonal blocks fuse causal masking into the clamp:
   (psum max 0) * mask01 -> int8.
 - Rows 0..511 (the large-magnitude early rows) use a bf16 path end-to-end:
   bf16 projections, bf16 scores + additive -1e30 mask, Act true exp -> bf16,
   bf16 attn@V and output projection.
 - V is scaled by 16 and Wo by 32 to dodge e4m3 subnormals; undone in the
   final PSUM->SBUF copies.
"""

import math
from contextlib import ExitStack

import numpy as np

import concourse.bass as bass
import concourse.mybir as mybir
import concourse.tile as tile
from concourse import bacc
from concourse._compat import with_exitstack
from concourse.bass import ds
from concourse.bass_utils import run_bass_kernel_spmd
from concourse.masks import make_identity

P = 128
F = 512            # q-chunk width
S = 4096
D = 2048
HD = 128
NB = S // F        # 8 chunks
NJB = S // P       # 32 key blocks
KOP = D // 256     # 8 DoubleRow contraction pairs for projections
F32 = mybir.dt.float32
BF16 = mybir.dt.bfloat16
FP8E4 = mybir.dt.float8e4
FP8E5 = mybir.dt.float8e5
I8 = mybir.dt.int8
DRM = mybir.MatmulPerfMode.DoubleRow
ExpF = mybir.ActivationFunctionType.Exp
CopyF = mybir.ActivationFunctionType.Copy
AL = mybir.AluOpType

GR = 4.0                                # e5m2 bits per octave
OFF = 60.0                              # e5m2 bits of 1.0
B0 = 66.0                               # recentering constant
CLIP = 123.0                            # max finite e5m2 bits
A_SCALE = GR / math.log(2.0)            # score prescale (in q tables)
ACT_SCALE = math.log(2.0) / GR
ACT_BIAS = -OFF * math.log(2.0) / GR
VSCALE = 16.0
WOSCALE = 32.0

# engine rotation patterns (A=Act, D=DVE, P=Pool/gpsimd)
EXP_PATTERN = "AD"         # non-diagonal fp8 exp ops (PSUM -> Act/DVE only)
DIAG_PATTERN = "DD"        # diagonal fp8 exp ops (tensor mask -> DVE)
OUT_PATTERN = "AAD"        # outproj psum->sbuf copies (PSUM -> Act/DVE only)


@with_exitstack
def _body(ctx: ExitStack, tc: tile.TileContext, io: dict):
    nc = tc.nc

    persist = ctx.enter_context(tc.tile_pool(name="persist", bufs=1))
    qT8 = persist.tile([65, 2, 2, S], FP8E4, tag="qT8")      # [p, t, h, i]
    kT8 = persist.tile([65, 2, S], FP8E4, tag="kT8")         # [p, t, j]
    vv8 = persist.tile([P, NJB // 2, 2, HD], FP8E4, tag="vv8")   # [j, jp, t, d]
    attnT8 = persist.tile([P, 2, S], FP8E4, tag="attnT8")    # [d, h, i]
    q16t = persist.tile([P, 2, F], BF16, tag="q16t")         # chunk-0 q
    kT16 = persist.tile([P, F], BF16, tag="kT16")            # chunk-0 k
    vv16 = persist.tile([P, 4, HD], BF16, tag="vv16")        # chunk-0 v
    attnT16 = persist.tile([P, 2, F], BF16, tag="attnT16")
    dmask = persist.tile([P, 4, F], F32, tag="dmask")
    m01 = persist.tile([P, 4, F], F32, tag="m01")
    bias0 = persist.tile([P, 4], F32, tag="bias0")
    ident32 = persist.tile([P, P], F32, tag="ident32")
    ones8 = persist.tile([P, 2, P], FP8E4, tag="ones8")
    ones16 = persist.tile([P, P], BF16, tag="ones16")
    abias = persist.tile([P, 1], F32, tag="abias")

    nc.sync.dma_start(dmask[:], io["dmask"])
    nc.sync.dma_start(m01[:], io["m01"])
    nc.sync.dma_start(bias0[:], io["bias0"])
    nc.sync.dma_start(kT8[64:65, :, :], io["brow"])
    nc.sync.dma_start(qT8[64:65, :, :, :], io["qones"])
    make_identity(nc, ident32[:])
    nc.vector.memset(ones8[:], 1.0)
    nc.vector.memset(ones16[:], 1.0)
    nc.vector.memset(abias[:], ACT_BIAS)

    wpool = ctx.enter_context(tc.tile_pool(name="wpool", bufs=1))
    wq8 = wpool.tile([P, KOP, 2, 256], FP8E4, tag="wq8")
    wk8 = wpool.tile([P, KOP, 2, HD], FP8E4, tag="wk8")
    wv8 = wpool.tile([P, KOP, 2, HD], FP8E4, tag="wv8")
    wq16 = wpool.tile([P, 16, 256], BF16, tag="wq16")
    wk16 = wpool.tile([P, 16, HD], BF16, tag="wk16")
    wv16 = wpool.tile([P, 16, HD], BF16, tag="wv16")
    wo8 = wpool.tile([P, 2, D], FP8E4, tag="wo8")
    wo16 = wpool.tile([P, 2, D], BF16, tag="wo16")
    xt16 = wpool.tile([P, 16, F], BF16, tag="xt16")
    for nm, t in [("wq8", wq8), ("wk8", wk8), ("wv8", wv8), ("wq16", wq16),
                  ("wk16", wk16), ("wv16", wv16), ("wo8", wo8), ("wo16", wo16),
                  ("xt16", xt16)]:
        nc.sync.dma_start(t[:], io[nm])

    xt_pool = ctx.enter_context(tc.tile_pool(name="xt", bufs=2))
    tab_pool = ctx.enter_context(tc.tile_pool(name="tab", bufs=2))
    rope_pool = ctx.enter_context(tc.tile_pool(name="rope", bufs=3))
    exp_pool = ctx.enter_context(tc.tile_pool(name="exp", bufs=4))
    bc_pool = ctx.enter_context(tc.tile_pool(name="bc", bufs=2))
    ob_pool = ctx.enter_context(tc.tile_pool(name="ob", bufs=2))
    psProj = ctx.enter_context(tc.tile_pool(name="psProj", bufs=2, space="PSUM"))
    psSc = ctx.enter_context(tc.tile_pool(name="psSc", bufs=3, space="PSUM"))
    psAv = ctx.enter_context(tc.tile_pool(name="psAv", bufs=1, space="PSUM"))
    psSum = ctx.enter_context(tc.tile_pool(name="psSum", bufs=1, space="PSUM"))
    psO = ctx.enter_context(tc.tile_pool(name="psO", bufs=1, space="PSUM"))

    rr = {"exp": 0, "diag": 0, "out": 0}

    def pick(kind, pattern):
        c = pattern[rr[kind] % len(pattern)]
        rr[kind] += 1
        return c

    def emit_exp_fp8(sc_ps, ex_slot, ex_slot_i8, dp):
        if dp >= 0:
            c = pick("diag", DIAG_PATTERN)
            eng = nc.vector if c == "D" else nc.gpsimd
            eng.scalar_tensor_tensor(ex_slot_i8, sc_ps, 0.0, m01[:, dp, :],
                                     op0=AL.max, op1=AL.mult)
        else:
            c = pick("exp", EXP_PATTERN)
            if c == "A":
                nc.scalar.activation(ex_slot, sc_ps, ExpF, bias=abias[:],
                                     scale=ACT_SCALE)
            else:
                eng = nc.vector if c == "D" else nc.gpsimd
                eng.tensor_scalar(ex_slot_i8, sc_ps, CLIP, 0.0,
                                  op0=AL.min, op1=AL.max)

    def rope(ps, ct, st, dest_ap):
        tmp = rope_pool.tile([P, F], F32, tag="tmp")
        nc.scalar.copy(tmp[:], ps[:])
        rotd = rope_pool.tile([P, F], F32, tag="rot")
        nc.gpsimd.dma_start(rotd[0:64, :], tmp[64:128, :])
        nc.gpsimd.dma_start(rotd[64:128, :], tmp[0:64, :])
        r2 = rope_pool.tile([P, F], F32, tag="r2")
        nc.gpsimd.tensor_tensor(r2[:], rotd[:], st, op=AL.mult)
        t1 = rope_pool.tile([P, F], F32, tag="t1")
        nc.gpsimd.tensor_tensor(t1[:], tmp[:], ct, op=AL.mult)
        nc.gpsimd.tensor_tensor(dest_ap, t1[:], r2[:], op=AL.add)

    def vtrans(psv, vv8_dst, vv16_dst):
        """v psum [d, i] -> x16, transpose per 128-block -> vv8 (and vv16)."""
        vt32 = rope_pool.tile([P, F], F32, tag="vt32")
        nc.scalar.activation(vt32[:], psv[:], CopyF, scale=VSCALE)
        ptf = psSc.tile([P, F], F32, tag="sc")
        for dp in range(4):
            nc.tensor.transpose(ptf[:, ds(dp * P, P)], vt32[:, ds(dp * P, P)],
                                ident32[:])
        nc.scalar.copy(vv8_dst, ptf[:])
        if vv16_dst is not None:
            nc.scalar.copy(vv16_dst, ptf[:])

    pending = []

    def drain_one():
        while pending:
            try:
                next(pending[0])
                return True
            except StopIteration:
                pending.pop(0)
        return False

    def flush_pending():
        while drain_one():
            pass

    def proj8(xt8c, w_sb, m0, width):
        ps = psProj.tile([P, F], F32, tag="pp")
        for kop in range(KOP):
            nc.tensor.matmul(ps[:], lhsT=w_sb[:, kop, :, ds(m0, width)],
                             rhs=xt8c[:, kop, :, :],
                             start=(kop == 0), stop=(kop == KOP - 1),
                             perf_mode=DRM)
        return ps

    def proj16(w_sb, m0, width):
        ps = psProj.tile([P, F], F32, tag="pp")
        for ko in range(16):
            nc.tensor.matmul(ps[:], lhsT=w_sb[:, ko, ds(m0, width)],
                             rhs=xt16[:, ko, :],
                             start=(ko == 0), stop=(ko == 15))
        return ps

    def proj_items(nb):
        """Generator emitting q/k/v projection work for chunk nb."""
        def gen():
            sl = ds(nb * F, F)
            tabs = tab_pool.tile([P, 4, F], BF16, tag="tabs")
            nc.sync.dma_start(tabs[:], io["tabs"][:, :, sl])
            ctA, stA = tabs[:, 0, :], tabs[:, 1, :]
            ct, st = tabs[:, 2, :], tabs[:, 3, :]
            if nb == 0:
                # kv were done in the prologue; only bf16 q remains
                yield
                for h in range(2):
                    psq = proj16(wq16, h * P, P)
                    rope(psq, ctA, stA, q16t[:, h, :])
                    yield
                return
            xt8c = xt_pool.tile([P, KOP, 2, F], FP8E4, tag="xt8c")
            nc.sync.dma_start(xt8c[:], io["xt8"][:, :, :, sl])
            yield
            for h in range(2):
                psq = proj8(xt8c, wq8, h * P, P)
                stq = rope_pool.tile([P, F], FP8E4, tag="stq8")
                rope(psq, ctA, stA, stq[:])
                nc.gpsimd.dma_start(qT8[0:64, 0, h, sl], stq[0:64, :])
                nc.gpsimd.dma_start(qT8[0:64, 1, h, sl], stq[64:128, :])
                yield
            psk = proj8(xt8c, wk8, 0, P)
            stk = rope_pool.tile([P, F], FP8E4, tag="stk8")
            rope(psk, ct, st, stk[:])
            nc.gpsimd.dma_start(kT8[0:64, 0, sl], stk[0:64, :])
            nc.gpsimd.dma_start(kT8[0:64, 1, sl], stk[64:128, :])
            yield
            psv = proj8(xt8c, wv8, 0, P)
            vtrans(psv, vv8[:, nb * 2: nb * 2 + 2, :, :], None)
            yield
        return gen()

    def outproj_items(nb):
        for ib in range(4):
            i2 = nb * 4 + ib
            ob = ob_pool.tile([P, D], BF16, tag="ob")

            def mk(i2=i2, ob=ob, first=(nb == 0)):
                def items():
                    for e in range(4):
                        po = psO.tile([P, F], F32, tag="po")
                        if first:
                            for hh in range(2):
                                nc.tensor.matmul(
                                    po[:], lhsT=attnT16[:, hh, ds((i2 % 4) * P, P)],
                                    rhs=wo16[:, hh, ds(e * F, F)],
                                    start=(hh == 0), stop=(hh == 1))
                            yield
                            nc.scalar.activation(ob[:, ds(e * F, F)], po[:],
                                                 CopyF, scale=1.0 / VSCALE)
                        else:
                            nc.tensor.matmul(
                                po[:], lhsT=attnT8[:, :, ds(i2 * P, P)],
                                rhs=wo8[:, :, ds(e * F, F)],
                                start=True, stop=True, perf_mode=DRM)
                            yield
                            c = pick("out", OUT_PATTERN)
                            sc = 1.0 / (VSCALE * WOSCALE)
                            if c == "A":
                                nc.scalar.activation(ob[:, ds(e * F, F)], po[:],
                                                     CopyF, scale=sc)
                            elif c == "D":
                                nc.vector.tensor_scalar_mul(
                                    ob[:, ds(e * F, F)], po[:], sc)
                            else:
                                nc.gpsimd.tensor_scalar_mul(
                                    ob[:, ds(e * F, F)], po[:], sc)
                        yield
                    nc.sync.dma_start(io["outp"][ds(i2 * P, P), :], ob[:])
                    yield
                return items()
            pending.append(mk())

    def attention_fp8(nb):
        sl = ds(nb * F, F)
        npair = (4 * nb + 4) // 2
        for h in range(2):
            av = psAv.tile([P, F], F32, tag="av")
            sm = psSum.tile([P, F], F32, tag="sm")

            def av_sum(ex_prev, jp_prev):
                nc.tensor.matmul(av[:], lhsT=vv8[:, jp_prev, :, :],
                                 rhs=ex_prev[:],
                                 start=(jp_prev == 0),
                                 stop=(jp_prev == npair - 1),
                                 perf_mode=DRM)
                nc.tensor.matmul(sm[:], lhsT=ones8[:], rhs=ex_prev[:],
                                 start=(jp_prev == 0),
                                 stop=(jp_prev == npair - 1),
                                 perf_mode=DRM)

            prev = None
            for jp in range(npair):
                ex = exp_pool.tile([P, 2, F], FP8E5, tag="ex")
                for t in range(2):
                    jb = 2 * jp + t
                    sc = psSc.tile([P, F], F32, tag="sc")
                    nc.tensor.matmul(sc[:], lhsT=kT8[:, :, ds(jb * P, P)],
                                     rhs=qT8[:, :, h, sl],
                                     start=True, stop=True, perf_mode=DRM)
                    emit_exp_fp8(sc[:], ex[:, t, :],
                                 ex[:, t, :].bitcast(I8), jb - 4 * nb)
                drain_one()
                drain_one()
                if prev is not None:
                    av_sum(*prev)
                prev = (ex, jp)
            av_sum(*prev)
            rbc = bc_pool.tile([P, F], F32, tag="rbc")
            nc.vector.reciprocal_approx_fast(rbc[:], sm[:])
            nc.vector.tensor_tensor(attnT8[:, h, sl], av[:], rbc[:],
                                    op=AL.mult)

    def attention_c0():
        for h in range(2):
            av = psAv.tile([P, F], F32, tag="av")
            sm = psSum.tile([P, F], F32, tag="sm")
            for jb in range(4):
                sc = psSc.tile([P, F], F32, tag="sc")
                nc.tensor.matmul(sc[:], lhsT=kT16[:, ds(jb * P, P)],
                                 rhs=q16t[:, h, :], start=True, stop=True)
                nc.vector.tensor_tensor(sc[:], sc[:], dmask[:, jb, :],
                                        op=AL.add)
                ex0 = exp_pool.tile([P, F], BF16, tag="ex0")
                nc.scalar.activation(ex0[:], sc[:], ExpF,
                                     bias=bias0[:, jb: jb + 1],
                                     scale=ACT_SCALE)
                nc.tensor.matmul(av[:], lhsT=vv16[:, jb, :], rhs=ex0[:],
                                 start=(jb == 0), stop=(jb == 3))
                nc.tensor.matmul(sm[:], lhsT=ones16[:], rhs=ex0[:],
                                 start=(jb == 0), stop=(jb == 3))
                drain_one()
            rbc = bc_pool.tile([P, F], F32, tag="rbc")
            nc.vector.reciprocal_approx_fast(rbc[:], sm[:])
            nc.vector.tensor_tensor(attnT16[:, h, :], av[:], rbc[:],
                                    op=AL.mult)

    # ── prologue: chunk-0 K/V (bf16 path; also feeds the fp8 tiles) ──
    tabs0 = tab_pool.tile([P, 4, F], BF16, tag="tabs")
    nc.sync.dma_start(tabs0[:], io["tabs"][:, :, ds(0, F)])
    psk = proj16(wk16, 0, P)
    rope(psk, tabs0[:, 2, :], tabs0[:, 3, :], kT16[:])
    stk8 = rope_pool.tile([P, F], FP8E4, tag="stk8")
    nc.vector.tensor_copy(stk8[:], kT16[:])
    nc.gpsimd.dma_start(kT8[0:64, 0, ds(0, F)], stk8[0:64, :])
    nc.gpsimd.dma_start(kT8[0:64, 1, ds(0, F)], stk8[64:128, :])
    psv = proj16(wv16, 0, P)
    vtrans(psv, vv8[:, 0:2, :, :], vv16[:])

    ORDER = [1, 2, 3, 4, 5, 6, 7, 0]
    pending.append(proj_items(ORDER[0]))
    for idx, nb in enumerate(ORDER):
        flush_pending()
        if idx + 1 < len(ORDER):
            pending.append(proj_items(ORDER[idx + 1]))
        if nb == 0:
            attention_c0()
        else:
            attention_fp8(nb)
        outproj_items(nb)
    flush_pending()


_NC_CACHE = None


def build_nc():
    global _NC_CACHE
    if _NC_CACHE is not None:
        return _NC_CACHE
    nc = bacc.Bacc("TRN2", target_bir_lowering=False, debug=False)
    io = {
        "xt8": nc.dram_tensor("xt8", [P, KOP, 2, S], FP8E4, kind="ExternalInput").ap(),
        "xt16": nc.dram_tensor("xt16", [P, 16, F], BF16, kind="ExternalInput").ap(),
        "tabs": nc.dram_tensor("tabs", [P, 4, S], BF16, kind="ExternalInput").ap(),
        "wq8": nc.dram_tensor("wq8", [P, KOP, 2, 256], FP8E4, kind="ExternalInput").ap(),
        "wk8": nc.dram_tensor("wk8", [P, KOP, 2, HD], FP8E4, kind="ExternalInput").ap(),
        "wv8": nc.dram_tensor("wv8", [P, KOP, 2, HD], FP8E4, kind="ExternalInput").ap(),
        "wq16": nc.dram_tensor("wq16", [P, 16, 256], BF16, kind="ExternalInput").ap(),
        "wk16": nc.dram_tensor("wk16", [P, 16, HD], BF16, kind="ExternalInput").ap(),
        "wv16": nc.dram_tensor("wv16", [P, 16, HD], BF16, kind="ExternalInput").ap(),
        "wo8": nc.dram_tensor("wo8", [P, 2, D], FP8E4, kind="ExternalInput").ap(),
        "wo16": nc.dram_tensor("wo16", [P, 2, D], BF16, kind="ExternalInput").ap(),
        "brow": nc.dram_tensor("brow", [1, 2, S], FP8E4, kind="ExternalInput").ap(),
        "qones": nc.dram_tensor("qones", [1, 2, 2, S], FP8E4, kind="ExternalInput").ap(),
        "bias0": nc.dram_tensor("bias0", [P, 4], F32, kind="ExternalInput").ap(),
        "dmask": nc.dram_tensor("dmask", [P, 4, F], F32, kind="ExternalInput").ap(),
        "m01": nc.dram_tensor("m01", [P, 4, F], F32, kind="ExternalInput").ap(),
        "outp": nc.dram_tensor("outp", [S, D], BF16, kind="ExternalOutput").ap(),
    }
    with tile.TileContext(nc) as tc:
        _body(tc, io)
    nc.compile()
    _NC_CACHE = nc
    return nc


def make_in_maps(hidden_states, attention_mask, cos, sin, gate, Wq, Wk, Wv, Wo):
    import ml_dtypes
    bf16 = ml_dtypes.bfloat16
    e4m3 = ml_dtypes.float8_e4m3
    f32 = np.float32

    X = np.asarray(hidden_states, f32).reshape(S, D)
    Xt = np.ascontiguousarray(X.T)                       # [D, S]
    xt8 = np.ascontiguousarray(
        Xt.reshape(KOP, 2, P, S).transpose(2, 0, 1, 3).astype(e4m3))
    xt16 = np.ascontiguousarray(
        X[:F].T.reshape(16, P, F).transpose(1, 0, 2).astype(bf16))

    cosT = np.asarray(cos, f32).reshape(S, HD).T         # [HD, S]
    sinT = np.asarray(sin, f32).reshape(S, HD).T
    sinTs = np.concatenate([-sinT[: HD // 2], sinT[HD // 2:]], axis=0)
    qs = f32(A_SCALE / math.sqrt(HD))
    tabs = np.ascontiguousarray(
        np.stack([cosT * qs, sinTs * qs, cosT, sinTs], axis=1).astype(bf16))

    g = np.asarray(gate, f32).reshape(S) + f32(1e-8)
    b = (B0 + GR * np.log2(g)).astype(f32)
    b_hi = b.astype(e4m3)
    b_lo = (b - b_hi.astype(f32)).astype(e4m3)
    brow = np.ascontiguousarray(np.stack([b_hi, b_lo], axis=0)[None])  # [1,2,S]
    qones = np.ones((1, 2, 2, S), dtype=e4m3)
    bias0 = np.ascontiguousarray(
        (np.log(g[:F]) + (B0 - OFF) * math.log(2.0) / GR)
        .reshape(4, P).T.astype(f32))                    # [128, 4]

    jj = np.arange(P)[:, None, None]
    pp = np.arange(4)[None, :, None] * P
    ii = np.arange(F)[None, None, :]
    valid = (jj + pp) <= ii
    dmask = np.ascontiguousarray(
        np.where(valid, f32(0), f32(-1e30)).astype(f32))
    m01 = np.ascontiguousarray(valid.astype(f32))

    Wq = np.asarray(Wq, f32); Wk = np.asarray(Wk, f32)
    Wv = np.asarray(Wv, f32); Wo = np.asarray(Wo, f32)

    def pack8(Wslice):               # [D, M] -> [128, KOP, 2, M]
        M = Wslice.shape[1]
        return np.ascontiguousarray(
            Wslice.reshape(KOP, 2, P, M).transpose(2, 0, 1, 3).astype(e4m3))

    def pack16(Wslice):              # [D, M] -> [128, 16, M]
        M = Wslice.shape[1]
        return np.ascontiguousarray(
            Wslice.reshape(16, P, M).transpose(1, 0, 2).astype(bf16))

    in_maps = []
    for c in range(8):
        gk = c // 2
        wq_sl = Wq[:, c * 256:(c + 1) * 256]
        wk_sl = Wk[:, gk * HD:(gk + 1) * HD]
        wv_sl = Wv[:, gk * HD:(gk + 1) * HD]
        wo_sl = Wo[c * 256:(c + 1) * 256, :]             # [256, D]
        wo8 = np.ascontiguousarray(
            (wo_sl * WOSCALE).reshape(2, P, D).transpose(1, 0, 2).astype(e4m3))
        wo16 = np.ascontiguousarray(
            wo_sl.reshape(2, P, D).transpose(1, 0, 2).astype(bf16))
        in_maps.append({
            "xt8": xt8, "xt16": xt16, "tabs": tabs,
            "wq8": pack8(wq_sl), "wk8": pack8(wk_sl), "wv8": pack8(wv_sl),
            "wq16": pack16(wq_sl), "wk16": pack16(wk_sl), "wv16": pack16(wv_sl),
            "wo8": wo8, "wo16": wo16,
            "brow": brow, "qones": qones, "bias0": bias0,
            "dmask": dmask, "m01": m01,
        })
    return in_maps


def kernel(hidden_states, attention_mask, cos, sin, gate, Wq, Wk, Wv, Wo,
           **kwargs):
    nc = build_nc()
    in_maps = make_in_maps(
        hidden_states, attention_mask, cos, sin, gate, Wq, Wk, Wv, Wo
    )
    res = run_bass_kernel_spmd(nc, in_maps, core_ids=list(range(8)), **kwargs)
    acc = res.results[0]["outp"].astype(np.float32)
    for c in range(1, 8):
        acc = acc + res.results[c]["outp"].astype(np.float32)
    out = acc.reshape(1, S, D)
    if kwargs:
        return out, res
    return out
